# revision 1
# baseline (speedup 1.0000x reference)
"""Trainium2 Bass kernel for nn_Decoder (pre-LN transformer decoder layer).

Sharding: 8 cores = 4 batches x 2 sequence-halves. Core pid -> (batch=pid//2,
s=pid%2). s=0 handles query tokens [0,T0), s=1 handles [T0,L). Each core
computes k/v for its key range on its own (s=1 recomputes the prefix
projections), so no collectives are needed; the host concatenates outputs.

Layout strategy: activations token-major for LN/residual (per-partition
scalars via bn_stats), feature-major (transposed via bf16 PE transpose) for
the matmul chain. Attention computed fully on-chip flash-style:
scoresT [keys, q] -> exp (ACT, scale=1/HD) -> PV matmul with 64 ones-columns
(shared tail block addressed via a strided AP) so the softmax denominator
lands in PSUM partitions 64:127. All matmuls bf16 (weights cast during DMA),
everything else fp32.
"""
import os
import sys

sys.path.insert(0, "/opt/trn_rl_repo")

import contextlib

import numpy as np

import concourse.bass as bass
import concourse.mybir as mybir
import concourse.tile as tile
import concourse.tile_utils as tile_utils
from concourse import bacc
from concourse.bass_utils import run_bass_kernel_spmd
from concourse.masks import make_identity

# trn2 has 224KB/partition physical, ~208 usable; default cap is stale 192.
tile_utils.max_sbuf_usage = 206 * 1024

F32 = mybir.dt.float32
BF16 = mybir.dt.bfloat16
AF = mybir.ActivationFunctionType
ALU = mybir.AluOpType

if os.environ.get("DECODER_DIMS"):
    B, L, D, H, I, T0 = (int(v) for v in os.environ["DECODER_DIMS"].split(","))
else:
    B, L, D, H, I, T0 = 4, 2048, 768, 12, 3072, 1280
HD = 64
T1 = L - T0
EPS = 1e-5
N_CORES = 2 * B
ND = D // 128
NI = I // 128
NH = H
MASK_BIG = -1e9
BN_SUB = 256


def attn_spans(q_start, q_len, span=512):
    out = []
    q0 = q_start
    while q0 < q_start + q_len:
        w = min(span, q_start + q_len - q0)
        out.append((q0, w, q0 // 128))
        q0 += w
    return out


def build_body(nc, tc, ctx, io, q_start, q_len, kv_len):
    x, w_qkv, w_o, w1, w2, bqkv, bo, b1, b2, g1, bl1, g2, bl2, out = io
    NT_KV = kv_len // 128
    NT_Q = q_len // 128

    # ---------------- constant tiles ----------------
    consts = ctx.enter_context(tc.tile_pool(name="consts", bufs=1))
    ident = consts.tile([128, 128], BF16, tag="ident")
    make_identity(nc, ident[:])
    causal = consts.tile([128, 128], F32, tag="causal")
    nc.vector.memset(causal[:], 0.0)
    nc.gpsimd.affine_select(
        out=causal[:], in_=causal[:], pattern=[[1, 128]],
        channel_multiplier=-1, base=0, compare_op=ALU.is_ge, fill=MASK_BIG)
    eps_t = consts.tile([128, 1], F32, tag="eps")
    nc.vector.memset(eps_t[:], EPS)

    def bcast(vec_ap, n, name, dtype=F32):
        t = consts.tile([128, n], dtype, tag=name)
        src = bass.AP(tensor=vec_ap.tensor, offset=vec_ap.offset,
                      ap=[[0, 128]] + vec_ap.ap)
        nc.gpsimd.dma_start(out=t[:], in_=src)
        return t

    g1_bc = bcast(g1, D, "g1_bc", BF16)
    bl1_bc = bcast(bl1, D, "bl1_bc", BF16)
    g2_bc = bcast(g2, D, "g2_bc", BF16)
    bl2_bc = bcast(bl2, D, "bl2_bc", BF16)
    bo_bc = bcast(bo, D, "bo_bc")          # f32: residual path
    b2_bc = bcast(b2, D, "b2_bc")          # f32: pre-gelu
    bv_src = bass.AP(tensor=bqkv.tensor, offset=bqkv.offset + 2 * HD,
                     ap=[[0, 128], [3 * HD, NH], [1, HD]])
    bv_bc = consts.tile([128, NH * HD], F32, tag="bv_bc")
    nc.gpsimd.dma_start(out=bv_bc[:], in_=bv_src)

    # Long-lived pools; stack (open order) must be reverse of close order:
    # closes: ht (after B), qkv (after C), wearly (after C), rest at end.
    oa_pool = ctx.enter_context(tc.tile_pool(name="oa_pool", bufs=5))
    h2_pool = ctx.enter_context(tc.tile_pool(name="h2_pool", bufs=2))
    at_cm = tc.tile_pool(name="attn_pool", bufs=1)
    at_pool = at_cm.__enter__()
    wearly_cm = tc.tile_pool(name="wearly", bufs=1)
    wearly = wearly_cm.__enter__()

    # ---------------- early weights (qkv) ----------------
    wqk = wearly.tile([128, ND, NH * 128], BF16, tag="wqk")
    for d in range(ND):
        src = bass.AP(tensor=w_qkv.tensor,
                      offset=w_qkv.offset + d * 128 * 3 * D,
                      ap=[[3 * D, 128], [3 * HD, NH], [1, 2 * HD]])
        nc.gpsimd.dma_start(out=wqk[:, d, :], in_=src)
    wv = wearly.tile([128, ND, NH * HD], BF16, tag="wv")
    for d in range(ND):
        src = bass.AP(tensor=w_qkv.tensor,
                      offset=w_qkv.offset + d * 128 * 3 * D + 2 * HD,
                      ap=[[3 * D, 128], [3 * HD, NH], [1, HD]])
        nc.gpsimd.dma_start(out=wv[:, d, :], in_=src)
    bqk = wearly.tile([128, NH], F32, tag="bqk")
    nc.gpsimd.dma_start(
        out=bqk[:],
        in_=bass.AP(tensor=bqkv.tensor, offset=bqkv.offset,
                    ap=[[1, 128], [3 * HD, NH]]))
    qkv_cm = tc.tile_pool(name="qkv_pool", bufs=1)
    qkv_pool = qkv_cm.__enter__()
    ht_cm = tc.tile_pool(name="ht_pool", bufs=1)
    ht_pool = ht_cm.__enter__()

    # ---------------- phase A: x -> LN1 -> h (bf16) -> hT ----------------
    hT = ht_pool.tile([128, ND, kv_len], BF16, tag="hT")

    def layernorm_tokmajor(x_t, g_bc, b_bc, pool, tagp):
        stats = pool.tile([128, D // BN_SUB, 6], F32, tag=tagp + "_stats")
        xs = x_t[:].rearrange("p (s c) -> p s c", c=BN_SUB)
        for sgi in range(D // BN_SUB):
            nc.vector.bn_stats(out=stats[:, sgi, :], in_=xs[:, sgi, :])
        mv = pool.tile([128, 2], F32, tag=tagp + "_mv")
        nc.vector.bn_aggr(out=mv[:], in_=stats[:])
        rstd = pool.tile([128, 1], F32, tag=tagp + "_rstd")
        nc.scalar.activation(out=rstd[:], in_=mv[:, 1:2],
                             func=AF.Sqrt, bias=eps_t[:], scale=1.0)
        nc.vector.reciprocal(out=rstd[:], in_=rstd[:])
        hc = pool.tile([128, D], BF16, tag=tagp + "_hc")
        nc.vector.tensor_scalar(
            out=hc[:], in0=x_t, scalar1=mv[:, 0:1], scalar2=rstd[:],
            op0=ALU.subtract, op1=ALU.mult)
        nc.vector.tensor_tensor(out=hc[:], in0=hc[:], in1=g_bc[:],
                                op=ALU.mult)
        hb = pool.tile([128, D], BF16, tag=tagp + "_hb")
        nc.vector.tensor_tensor(out=hb[:], in0=hc[:], in1=b_bc[:],
                                op=ALU.add)
        return hb

    with contextlib.ExitStack() as phA:
        xpool = phA.enter_context(tc.tile_pool(name="xpool", bufs=3))
        lnp = phA.enter_context(tc.tile_pool(name="lnp", bufs=2))
        tpp = phA.enter_context(
            tc.tile_pool(name="tpp", bufs=4, space="PSUM"))
        for tw in range(NT_KV):
            x_t = xpool.tile([128, D], F32, tag="x_t")
            nc.sync.dma_start(out=x_t[:], in_=x[tw * 128:(tw + 1) * 128, :])
            hb = layernorm_tokmajor(x_t[:], g1_bc, bl1_bc, lnp, "ln1")
            for d in range(ND):
                pt = tpp.tile([128, 128], BF16, tag="tp_ps")
                nc.tensor.transpose(pt[:], hb[:, d * 128:(d + 1) * 128],
                                    ident[:])
                nc.vector.tensor_copy(
                    out=hT[:, d, tw * 128:(tw + 1) * 128], in_=pt[:])

    # ---------------- phase B: qkv projections ----------------
    # head h at partition half 64*(h%2), pair h//2, in both qT2 and kT2.
    qT2 = qkv_pool.tile([128, NH // 2, q_len], BF16, tag="qT2")
    kT2 = qkv_pool.tile([128, NH // 2, kv_len], BF16, tag="kT2")
    # v token-major: per (tok-window, head) a [128, 128] block of
    # [v (64 cols) | ones (64 cols)] so the PV matmul also produces the
    # softmax denominator on PSUM partitions 64:127.
    vaug = qkv_pool.tile([128, NT_KV, NH, 128], BF16, tag="vaug")

    def vaug_lhsT(kt, h):
        return vaug[:, kt, h, :]

    with contextlib.ExitStack() as phB:
        qkps = phB.enter_context(
            tc.tile_pool(name="qkps", bufs=2, space="PSUM"))
        vps = phB.enter_context(tc.tile_pool(name="vps", bufs=2, space="PSUM"))

        for sp0 in range(0, kv_len, 512):
            w = min(512, kv_len - sp0)
            for h in range(NH):
                hb2, hp = 64 * (h % 2), h // 2
                pq = qkps.tile([128, 512], F32, tag="pqk")
                for d in range(ND):
                    nc.tensor.matmul(
                        pq[:, 0:w], wqk[:, d, h * 128:(h + 1) * 128],
                        hT[:, d, sp0:sp0 + w],
                        start=(d == 0), stop=(d == ND - 1))
                nc.vector.tensor_scalar_add(
                    out=kT2[hb2:hb2 + 64, hp, sp0:sp0 + w],
                    in0=pq[64:128, 0:w], scalar1=bqk[64:128, h:h + 1])
                lo = max(sp0, q_start)
                hi = min(sp0 + w, q_start + q_len)
                if lo < hi:
                    nc.vector.tensor_scalar_add(
                        out=qT2[hb2:hb2 + 64, hp, lo - q_start:hi - q_start],
                        in0=pq[0:64, lo - sp0:hi - sp0],
                        scalar1=bqk[0:64, h:h + 1])
        half = NH * HD // 2
        nhh = NH // 2
        for tw in range(NT_KV):
            for hf in range(2):
                pv = vps.tile([128, half], F32, tag="pv")
                for d in range(ND):
                    nc.tensor.matmul(
                        pv[:], hT[:, d, tw * 128:(tw + 1) * 128],
                        wv[:, d, hf * half:(hf + 1) * half],
                        start=(d == 0), stop=(d == ND - 1))
                dst = vaug[:, tw, hf * nhh:(hf + 1) * nhh, 0:HD]
                bvs = bv_bc[:, hf * half:(hf + 1) * half].rearrange(
                    "p (h c) -> p h c", c=HD)
                nc.vector.tensor_tensor(
                    out=dst, in0=pv[:].rearrange("p (h c) -> p h c", c=HD),
                    in1=bvs, op=ALU.add)
            nc.vector.memset(vaug[:, tw, :, HD:128], 1.0)

    ht_cm.__exit__(None, None, None)

    # ---------------- phase C: attention ----------------
    attnT = at_pool.tile([128, ND, q_len], BF16, tag="attnT")

    with contextlib.ExitStack() as phC:
        sps = phC.enter_context(tc.tile_pool(name="sps", bufs=3, space="PSUM"))
        ops_ = phC.enter_context(
            tc.tile_pool(name="ops", bufs=2, space="PSUM"))
        epool = phC.enter_context(tc.tile_pool(name="epool", bufs=4))
        rpool = phC.enter_context(tc.tile_pool(name="rpool", bufs=2))

        for h in range(NH):
            hb2, hp = 64 * (h % 2), h // 2
            for (q0, w, nfull) in attn_spans(q_start, q_len):
                po = ops_.tile([128, 512], F32, tag="po")
                ndiag = w // 128
                ktot = nfull + ndiag

                def scores_mm(ps_slice, kt, c0):
                    nc.tensor.matmul(
                        ps_slice,
                        kT2[hb2:hb2 + 64, hp, kt * 128:(kt + 1) * 128],
                        qT2[hb2:hb2 + 64, hp,
                            q0 + c0 - q_start:q0 + w - q_start],
                        start=True, stop=True)

                def pv_mm(kt, et_slice, c0):
                    nc.tensor.matmul(
                        po[:, c0:w], vaug[:, kt, h, :], et_slice,
                        start=(kt == 0), stop=(kt == ktot - 1))

                # full key-tiles, two per PSUM bank (bf16 scores) so each
                # ACT exp covers 2*w elements (amortizes the 352-cyc fixed
                # ACTIVATE overhead)
                kt = 0
                while kt < nfull:
                    npair = min(2, nfull - kt)
                    ps = sps.tile([128, 2, 512], F32, tag="ps")
                    for jj in range(npair):
                        scores_mm(ps[:, jj, 0:w], kt + jj, 0)
                    et = epool.tile([128, 2, 512], BF16, tag="et")
                    nc.scalar.activation(out=et[:, 0:npair, 0:w],
                                         in_=ps[:, 0:npair, 0:w],
                                         func=AF.Exp, bias=0.0, scale=1.0 / HD)
                    for jj in range(npair):
                        pv_mm(kt + jj, et[:, jj, 0:w], 0)
                    kt += npair
                # diagonal key-tiles: causal mask added on the first 128
                # query columns before exp
                for diag_j in range(ndiag):
                    kt = nfull + diag_j
                    c0 = 128 * diag_j
                    wj = w - c0
                    ps = sps.tile([128, 2, 512], F32, tag="ps")
                    scores_mm(ps[:, 0, 0:wj], kt, c0)
                    nc.vector.tensor_tensor(
                        out=ps[:, 0, 0:128], in0=ps[:, 0, 0:128],
                        in1=causal[:], op=ALU.add)
                    et = epool.tile([128, 2, 512], BF16, tag="et")
                    nc.scalar.activation(out=et[:, 0, 0:wj], in_=ps[:, 0, 0:wj],
                                         func=AF.Exp, bias=0.0, scale=1.0 / HD)
                    pv_mm(kt, et[:, 0, 0:wj], c0)
                # 1/den = (1/sqrt(den))^2: rsqrt on ACT (table op), square
                # folded into the two cheap DVE multiplies; iterative DVE
                # reciprocal costs ~6.6 cycles/elem.
                rt = rpool.tile([64, 512], F32, tag="rt")
                nc.scalar.activation(out=rt[:, 0:w], in_=po[64:128, 0:w],
                                     func=AF.Abs_reciprocal_sqrt,
                                     bias=0.0, scale=1.0)
                rt2 = rpool.tile([64, 512], F32, tag="rt2")
                nc.vector.tensor_tensor(out=rt2[:, 0:w], in0=rt[:, 0:w],
                                        in1=rt[:, 0:w], op=ALU.mult)
                nc.vector.tensor_tensor(
                    out=attnT[64 * (h % 2):64 * (h % 2) + 64, hp,
                              q0 - q_start:q0 - q_start + w],
                    in0=po[0:64, 0:w], in1=rt2[:, 0:w], op=ALU.mult)

    qkv_cm.__exit__(None, None, None)
    wearly_cm.__exit__(None, None, None)

    # ------------- phases D+E interleaved per 512-token chunk -------------
    wffn_cm = tc.tile_pool(name="wffn", bufs=1)
    wffn = wffn_cm.__enter__()
    wo_sb = wffn.tile([128, ND, D], BF16, tag="wo_sb")
    for a in range(ND):
        nc.gpsimd.dma_start(out=wo_sb[:, a, :],
                            in_=w_o[a * 128:(a + 1) * 128, :])
    w1_sb = wffn.tile([128, ND, I], BF16, tag="w1_sb")
    for d in range(ND):
        nc.gpsimd.dma_start(out=w1_sb[:, d, :],
                            in_=w1[d * 128:(d + 1) * 128, :])
    w2_sb = wffn.tile([128, NI, D], BF16, tag="w2_sb")
    for i_ in range(NI):
        nc.gpsimd.dma_start(out=w2_sb[:, i_, :],
                            in_=w2[i_ * 128:(i_ + 1) * 128, :])
    b1_sb = wffn.tile([128, NI], F32, tag="b1_sb")
    nc.gpsimd.dma_start(
        out=b1_sb[:],
        in_=bass.AP(tensor=b1.tensor, offset=b1.offset,
                    ap=[[1, 128], [128, NI]]))

    with contextlib.ExitStack() as phDE:
        wops = phDE.enter_context(
            tc.tile_pool(name="wops", bufs=2, space="PSUM"))
        tpp2 = phDE.enter_context(
            tc.tile_pool(name="tpp2", bufs=2, space="PSUM"))
        xpool2 = phDE.enter_context(tc.tile_pool(name="xpool2", bufs=2))
        lnp2 = phDE.enter_context(tc.tile_pool(name="lnp2", bufs=2))
        f1ps = phDE.enter_context(
            tc.tile_pool(name="f1ps", bufs=2, space="PSUM"))
        f2ps = phDE.enter_context(
            tc.tile_pool(name="f2ps", bufs=2, space="PSUM"))
        opool = phDE.enter_context(tc.tile_pool(name="opool", bufs=2))
        Dh = D // 2

        for c0 in range(0, q_len, 512):
            cw = min(512, q_len - c0)
            ctws = cw // 128
            h2T = h2_pool.tile([128, ND, 512], BF16, tag="h2T")
            oa_tiles = []
            # --- phase D for this chunk: w_o + residual + LN2 + transpose
            for twl in range(ctws):
                tw = c0 // 128 + twl
                xo = xpool2.tile([128, D], F32, tag="xo")
                nc.sync.dma_start(
                    out=xo[:],
                    in_=x[q_start + tw * 128:q_start + (tw + 1) * 128, :])
                nc.vector.tensor_tensor(out=xo[:], in0=xo[:], in1=bo_bc[:],
                                        op=ALU.add)
                oa = oa_pool.tile([128, D], F32, tag="oa")
                oa_tiles.append(oa)
                for hf in range(2):
                    pw = wops.tile([128, Dh], F32, tag="pw")
                    for a in range(ND):
                        nc.tensor.matmul(
                            pw[:], attnT[:, a, tw * 128:(tw + 1) * 128],
                            wo_sb[:, a, hf * Dh:(hf + 1) * Dh],
                            start=(a == 0), stop=(a == ND - 1))
                    nc.vector.tensor_tensor(
                        out=oa[:, hf * Dh:(hf + 1) * Dh], in0=pw[:],
                        in1=xo[:, hf * Dh:(hf + 1) * Dh], op=ALU.add)
                hb = layernorm_tokmajor(oa[:], g2_bc, bl2_bc, lnp2, "ln2")
                for d in range(ND):
                    pt = tpp2.tile([128, 128], BF16, tag="tp2_ps")
                    nc.tensor.transpose(pt[:], hb[:, d * 128:(d + 1) * 128],
                                        ident[:])
                    nc.vector.tensor_copy(
                        out=h2T[:, d, twl * 128:(twl + 1) * 128], in_=pt[:])
            # --- phase E for this chunk: FFN + gelu + residual -> out
            ff1 = wffn.tile([128, NI, 512], BF16, tag="ff1")
            for i_ in range(NI):
                pf = f1ps.tile([128, 512], F32, tag="pf1")
                for d in range(ND):
                    nc.tensor.matmul(
                        pf[:, 0:cw], w1_sb[:, d, i_ * 128:(i_ + 1) * 128],
                        h2T[:, d, 0:cw],
                        start=(d == 0), stop=(d == ND - 1))
                nc.vector.tensor_scalar_add(
                    out=ff1[:, i_, 0:cw], in0=pf[:, 0:cw],
                    scalar1=b1_sb[:, i_:i_ + 1])
            for twl in range(ctws):
                tw = c0 // 128 + twl
                ot = opool.tile([128, D], F32, tag="ot")
                for hf in range(2):
                    pg = f2ps.tile([128, Dh], F32, tag="pf2")
                    for i_ in range(NI):
                        nc.tensor.matmul(
                            pg[:], ff1[:, i_, twl * 128:(twl + 1) * 128],
                            w2_sb[:, i_, hf * Dh:(hf + 1) * Dh],
                            start=(i_ == 0), stop=(i_ == NI - 1))
                    sl = slice(hf * Dh, (hf + 1) * Dh)
                    gb = opool.tile([128, Dh], F32, tag="gb")
                    nc.vector.tensor_tensor(out=gb[:], in0=pg[:],
                                            in1=b2_bc[:, sl], op=ALU.add)
                    nc.scalar.activation(out=gb[:], in_=gb[:], func=AF.Gelu,
                                         bias=0.0, scale=1.0)
                    nc.vector.tensor_tensor(out=ot[:, sl], in0=gb[:],
                                            in1=oa_tiles[twl][:, sl],
                                            op=ALU.add)
                nc.sync.dma_start(out=out[tw * 128:(tw + 1) * 128, :],
                                  in_=ot[:])

    wffn_cm.__exit__(None, None, None)
    at_cm.__exit__(None, None, None)


_NC_CACHE = {}


def build_kernel():
    key = (B, L, D, H, I, T0)
    if key in _NC_CACHE:
        return _NC_CACHE[key]
    nc = bacc.Bacc("TRN2", target_bir_lowering=False, debug=False,
                   num_devices=N_CORES)
    x = nc.dram_tensor("x", [L, D], F32, kind="ExternalInput").ap()
    w_qkv = nc.dram_tensor("w_qkv", [D, 3 * D], F32, kind="ExternalInput").ap()
    w_o = nc.dram_tensor("w_o", [D, D], F32, kind="ExternalInput").ap()
    w1 = nc.dram_tensor("w1", [D, I], F32, kind="ExternalInput").ap()
    w2 = nc.dram_tensor("w2", [I, D], F32, kind="ExternalInput").ap()
    bqkv = nc.dram_tensor("b_qkv", [3 * D], F32, kind="ExternalInput").ap()
    bo = nc.dram_tensor("b_o", [D], F32, kind="ExternalInput").ap()
    b1 = nc.dram_tensor("b1", [I], F32, kind="ExternalInput").ap()
    b2 = nc.dram_tensor("b2", [D], F32, kind="ExternalInput").ap()
    g1 = nc.dram_tensor("ln1_g", [D], F32, kind="ExternalInput").ap()
    bl1 = nc.dram_tensor("ln1_b", [D], F32, kind="ExternalInput").ap()
    g2 = nc.dram_tensor("ln2_g", [D], F32, kind="ExternalInput").ap()
    bl2 = nc.dram_tensor("ln2_b", [D], F32, kind="ExternalInput").ap()
    out = nc.dram_tensor("out", [T0, D], F32, kind="ExternalOutput").ap()
    io = (x, w_qkv, w_o, w1, w2, bqkv, bo, b1, b2, g1, bl1, g2, bl2, out)

    pid = nc.partition_id()
    with tile.TileContext(nc) as tc:
        with tc.If(pid % 2 == 0):
            with contextlib.ExitStack() as c0:
                build_body(nc, tc, c0, io, 0, T0, T0)
        with tc.If(pid % 2 == 1):
            with contextlib.ExitStack() as c1:
                build_body(nc, tc, c1, io, T0, T1, L)
    nc.compile()
    _NC_CACHE[key] = nc
    return nc


def kernel(**inputs):
    x = np.asarray(inputs["x"], dtype=np.float32)
    am = np.asarray(inputs["attention_mask"])
    assert am.all(), "kernel assumes attention_mask all-True (spec fill=ones)"
    names = ["w_qkv", "b_qkv", "w_o", "b_o", "w1", "b1", "w2", "b2",
             "ln1_g", "ln1_b", "ln2_g", "ln2_b"]
    common = {n: np.ascontiguousarray(np.asarray(inputs[n], np.float32))
              for n in names}
    nc = build_kernel()
    in_maps = []
    for pid in range(N_CORES):
        b = pid // 2
        m = dict(common)
        m["x"] = np.ascontiguousarray(x[b])
        in_maps.append(m)
    res = run_bass_kernel_spmd(nc, in_maps, core_ids=list(range(N_CORES)))
    out = np.empty((B, L, D), np.float32)
    for b in range(B):
        out[b, :T0] = res.results[2 * b]["out"][:T0]
        out[b, T0:] = res.results[2 * b + 1]["out"][:T1]
    return out


if __name__ == "__main__":
    rng = np.random.default_rng(0)
    ins = {
        "x": rng.standard_normal((B, L, D)).astype(np.float32),
        "attention_mask": np.ones((B, L), bool),
        "ln1_g": np.ones(D, np.float32), "ln1_b": np.zeros(D, np.float32),
        "w_qkv": (rng.standard_normal((D, 3 * D)) * 0.02).astype(np.float32),
        "b_qkv": np.zeros(3 * D, np.float32),
        "w_o": (rng.standard_normal((D, D)) * 0.02).astype(np.float32),
        "b_o": np.zeros(D, np.float32),
        "ln2_g": np.ones(D, np.float32), "ln2_b": np.zeros(D, np.float32),
        "w1": (rng.standard_normal((D, I)) * 0.02).astype(np.float32),
        "b1": np.zeros(I, np.float32),
        "w2": (rng.standard_normal((I, D)) * 0.02).astype(np.float32),
        "b2": np.zeros(D, np.float32),
    }
    o = kernel(**ins)
    print("kernel out:", o.shape, o.dtype, np.abs(o).max())



# revision 4
# speedup vs baseline: 1.1625x; 1.1625x over previous
"""Trainium2 Bass kernel for nn_Decoder (pre-LN transformer decoder layer).

Sharding: 8 cores = 4 batches x 2 sequence-halves. Core pid -> (batch=pid//2,
s=pid%2). s=0 handles query tokens [0,T0), s=1 handles [T0,L). Each core
computes k/v for its key range on its own (s=1 recomputes the prefix
projections), so no collectives are needed; the host concatenates outputs.

Layout strategy: activations token-major for LN/residual (per-partition
scalars via bn_stats), feature-major (transposed via bf16 PE transpose) for
the matmul chain. Attention computed fully on-chip flash-style:
scoresT [keys, q] -> exp (ACT, scale=1/HD) -> PV matmul whose lhsT pairs the
64 v-columns with a shared 64-wide ones block (strided AP) so the softmax
denominator lands in the other PSUM partition half. All matmuls bf16
(weights cast during DMA), everything else fp32.

ACT-table discipline: the whole kernel uses only Exp/Ln/Identity (set
natural_log_exp_and_others) plus one Gelu set load at the end. LN rstd is
exp(-0.5*ln(var+eps)); the softmax denominator reciprocal is exp(-ln(den));
causal masking multiplies the exp'd scores by a 0/1 bf16 mask. FFN weights
prefetch during attention (w_o+w1) and phase D (w2) so the PE never waits.
"""
import os
import sys

sys.path.insert(0, "/opt/trn_rl_repo")

import contextlib

import numpy as np

import concourse.bass as bass
import concourse.mybir as mybir
import concourse.tile as tile
import concourse.tile_utils as tile_utils
from concourse import bacc
from concourse.bass_utils import run_bass_kernel_spmd
from concourse.masks import make_identity

# trn2 has 224KB/partition physical, ~208 usable; default cap is stale 192.
tile_utils.max_sbuf_usage = 206 * 1024

F32 = mybir.dt.float32
BF16 = mybir.dt.bfloat16
AF = mybir.ActivationFunctionType
ALU = mybir.AluOpType

# The stock table chooser picks the first set containing each activation fn,
# so Exp->exp_and_others and Ln->natural_log thrash (2.7us per switch).
# Restrict every other set's view of Exp/Ln so the table-load pass resolves
# both to natural_log_exp_and_others (canonical set ids preserved).
_orig_gat = bacc.get_activation_tables


def _gat_exp_ln_merged(arch):
    t = dict(_orig_gat(arch))
    for name in list(t):
        if name != "natural_log_exp_and_others":
            t[name] = t[name] - {AF.Exp, AF.Ln}
    return t


bacc.get_activation_tables = _gat_exp_ln_merged

if os.environ.get("DECODER_DIMS"):
    B, L, D, H, I, T0 = (int(v) for v in os.environ["DECODER_DIMS"].split(","))
else:
    B, L, D, H, I, T0 = 4, 2048, 768, 12, 3072, 1280
HD = 64
T1 = L - T0
EPS = 1e-5
N_CORES = 2 * B
ND = D // 128
NI = I // 128
NH = H


def attn_spans(q_start, q_len, span=512):
    out = []
    q0 = q_start
    while q0 < q_start + q_len:
        w = min(span, q_start + q_len - q0)
        out.append((q0, w, q0 // 128))
        q0 += w
    return out


BN_SUB = 256


def build_body(nc, tc, ctx, io, q_start, q_len, kv_len):
    x, w_qkv, w_o, w1, w2, bqkv, bo, b1, b2, g1, bl1, g2, bl2, out = io
    NT_KV = kv_len // 128
    NT_Q = q_len // 128

    # ---------------- constant tiles ----------------
    consts = ctx.enter_context(tc.tile_pool(name="consts", bufs=1))
    ident = consts.tile([128, 128], BF16, tag="ident")
    make_identity(nc, ident[:])
    # 0/1 causal mask (bf16): causal01[k, c] = 1 if c >= k else 0
    causal01 = consts.tile([128, 128], BF16, tag="causal01")
    nc.vector.memset(causal01[:], 1.0)
    nc.gpsimd.affine_select(
        out=causal01[:], in_=causal01[:], pattern=[[1, 128]],
        channel_multiplier=-1, base=0, compare_op=ALU.is_ge, fill=0.0)
    eps_t = consts.tile([128, 1], F32, tag="eps")
    nc.vector.memset(eps_t[:], EPS)

    def bcast(vec_ap, n, name, dtype=F32):
        t = consts.tile([128, n], dtype, tag=name)
        src = bass.AP(tensor=vec_ap.tensor, offset=vec_ap.offset,
                      ap=[[0, 128]] + vec_ap.ap)
        nc.gpsimd.dma_start(out=t[:], in_=src)
        return t

    g1_bc = bcast(g1, D, "g1_bc", BF16)
    bl1_bc = bcast(bl1, D, "bl1_bc", BF16)
    g2_bc = bcast(g2, D, "g2_bc", BF16)
    bl2_bc = bcast(bl2, D, "bl2_bc", BF16)
    bo_bc = bcast(bo, D, "bo_bc")          # f32: residual path
    b2_bc = bcast(b2, D, "b2_bc")          # f32: pre-gelu
    bv_src = bass.AP(tensor=bqkv.tensor, offset=bqkv.offset + 2 * HD,
                     ap=[[0, 128], [3 * HD, NH], [1, HD]])
    bv_bc = consts.tile([128, NH * HD], F32, tag="bv_bc")
    nc.gpsimd.dma_start(out=bv_bc[:], in_=bv_src)

    # Long-lived pools; stack (open order) must be reverse of close order:
    # closes: ht (after B), qkv (after C), wearly (after C), wffnB/att at end.
    at_pool = ctx.enter_context(tc.tile_pool(name="attn_pool", bufs=1))
    wffnA = ctx.enter_context(tc.tile_pool(name="wffnA", bufs=1))
    wearly_cm = tc.tile_pool(name="wearly", bufs=1)
    wearly = wearly_cm.__enter__()

    # ---------------- early weights (qkv) ----------------
    wqk = wearly.tile([128, ND, NH * 128], BF16, tag="wqk")
    for d in range(ND):
        src = bass.AP(tensor=w_qkv.tensor,
                      offset=w_qkv.offset + d * 128 * 3 * D,
                      ap=[[3 * D, 128], [3 * HD, NH], [1, 2 * HD]])
        nc.gpsimd.dma_start(out=wqk[:, d, :], in_=src)
    wv = wearly.tile([128, ND, NH * HD], BF16, tag="wv")
    for d in range(ND):
        src = bass.AP(tensor=w_qkv.tensor,
                      offset=w_qkv.offset + d * 128 * 3 * D + 2 * HD,
                      ap=[[3 * D, 128], [3 * HD, NH], [1, HD]])
        nc.gpsimd.dma_start(out=wv[:, d, :], in_=src)
    bqk = wearly.tile([128, NH], F32, tag="bqk")
    nc.gpsimd.dma_start(
        out=bqk[:],
        in_=bass.AP(tensor=bqkv.tensor, offset=bqkv.offset,
                    ap=[[1, 128], [3 * HD, NH]]))
    qkv_cm = tc.tile_pool(name="qkv_pool", bufs=1)
    qkv_pool = qkv_cm.__enter__()
    ht_cm = tc.tile_pool(name="ht_pool", bufs=1)
    ht_pool = ht_cm.__enter__()

    # ---------------- phase A: x -> LN1 -> h (bf16) -> hT ----------------
    hT = ht_pool.tile([128, ND, kv_len], BF16, tag="hT")

    def layernorm_tokmajor(x_t, g_bc, b_bc, pool, tagp):
        stats = pool.tile([128, D // BN_SUB, 6], F32, tag=tagp + "_stats")
        xs = x_t.rearrange("p (s c) -> p s c", c=BN_SUB)
        for sgi in range(D // BN_SUB):
            nc.vector.bn_stats(out=stats[:, sgi, :], in_=xs[:, sgi, :])
        mv = pool.tile([128, 2], F32, tag=tagp + "_mv")
        nc.vector.bn_aggr(out=mv[:], in_=stats[:])
        # rstd = (var+eps)^-0.5 = exp(-0.5*ln(var+eps)): stays in the
        # natural_log_exp table set (no sqrt-set load).
        lnv = pool.tile([128, 1], F32, tag=tagp + "_lnv")
        nc.scalar.activation(out=lnv[:], in_=mv[:, 1:2],
                             func=AF.Ln, bias=eps_t[:], scale=1.0)
        rstd = pool.tile([128, 1], F32, tag=tagp + "_rstd")
        nc.scalar.activation(out=rstd[:], in_=lnv[:],
                             func=AF.Exp, bias=0.0, scale=-0.5)
        hc = pool.tile([128, D], BF16, tag=tagp + "_hc")
        nc.vector.tensor_scalar(
            out=hc[:], in0=x_t, scalar1=mv[:, 0:1], scalar2=rstd[:],
            op0=ALU.subtract, op1=ALU.mult)
        nc.vector.tensor_tensor(out=hc[:], in0=hc[:], in1=g_bc[:],
                                op=ALU.mult)
        hb = pool.tile([128, D], BF16, tag=tagp + "_hb")
        nc.vector.tensor_tensor(out=hb[:], in0=hc[:], in1=b_bc[:],
                                op=ALU.add)
        return hb

    with contextlib.ExitStack() as phA:
        xpool = phA.enter_context(tc.tile_pool(name="xpool", bufs=2))
        lnp = phA.enter_context(tc.tile_pool(name="lnp", bufs=2))
        tpp = phA.enter_context(
            tc.tile_pool(name="tpp", bufs=4, space="PSUM"))
        for tw in range(NT_KV):
            x_t = xpool.tile([128, D], F32, tag="x_t")
            nc.sync.dma_start(out=x_t[:], in_=x[tw * 128:(tw + 1) * 128, :])
            hb = layernorm_tokmajor(x_t[:], g1_bc, bl1_bc, lnp, "ln1")
            for d in range(ND):
                pt = tpp.tile([128, 128], BF16, tag="tp_ps")
                nc.tensor.transpose(pt[:], hb[:, d * 128:(d + 1) * 128],
                                    ident[:])
                nc.vector.tensor_copy(
                    out=hT[:, d, tw * 128:(tw + 1) * 128], in_=pt[:])

    # ---------------- phase B: qkv projections ----------------
    # head h at partition half 64*(h%2), pair h//2, in both qT2 and kT2.
    qT2 = qkv_pool.tile([128, NH // 2, q_len], BF16, tag="qT2")
    kT2 = qkv_pool.tile([128, NH // 2, kv_len], BF16, tag="kT2")
    # v token-major, pair-interleaved: per (key-tile, head-pair) a 192-col
    # block [v_even(64) | ones(64) | v_odd(64)]. The PV lhsT for the even
    # head is the contiguous window [v|ones] (attn -> PSUM partitions 0:63,
    # denominator -> 64:127); the odd head's window [ones|v] flips the
    # halves. One shared ones stripe per pair instead of one per head.
    NP = NH // 2
    vv3 = qkv_pool.tile([128, NT_KV, NP, 3 * HD], BF16, tag="vv3")
    nc.vector.memset(vv3[:, :, :, HD:2 * HD], 1.0)

    def vaug_lhsT(kt, h):
        j = h // 2
        if h % 2 == 0:
            return vv3[:, kt, j, 0:2 * HD]
        return vv3[:, kt, j, HD:3 * HD]

    with contextlib.ExitStack() as phB:
        qkps = phB.enter_context(
            tc.tile_pool(name="qkps", bufs=2, space="PSUM"))
        vps = phB.enter_context(tc.tile_pool(name="vps", bufs=2, space="PSUM"))

        for sp0 in range(0, kv_len, 512):
            w = min(512, kv_len - sp0)
            for h in range(NH):
                hb2, hp = 64 * (h % 2), h // 2
                pq = qkps.tile([128, 512], F32, tag="pqk")
                for d in range(ND):
                    nc.tensor.matmul(
                        pq[:, 0:w], wqk[:, d, h * 128:(h + 1) * 128],
                        hT[:, d, sp0:sp0 + w],
                        start=(d == 0), stop=(d == ND - 1))
                # k-evac on the ACT engine (idle in phase B), q on DVE.
                nc.scalar.activation(
                    out=kT2[hb2:hb2 + 64, hp, sp0:sp0 + w],
                    in_=pq[64:128, 0:w], func=AF.Identity,
                    bias=bqk[64:128, h:h + 1], scale=1.0)
                lo = max(sp0, q_start)
                hi = min(sp0 + w, q_start + q_len)
                if lo < hi:
                    nc.vector.tensor_scalar_add(
                        out=qT2[hb2:hb2 + 64, hp, lo - q_start:hi - q_start],
                        in0=pq[0:64, lo - sp0:hi - sp0],
                        scalar1=bqk[0:64, h:h + 1])
        half = NH * HD // 2
        for tw in range(NT_KV):
            for hf in range(2):
                pv = vps.tile([128, half], F32, tag="pv")
                for d in range(ND):
                    nc.tensor.matmul(
                        pv[:], hT[:, d, tw * 128:(tw + 1) * 128],
                        wv[:, d, hf * half:(hf + 1) * half],
                        start=(d == 0), stop=(d == ND - 1))
                # scatter the 6 heads into the pair-interleaved vv3 blocks:
                # even heads -> col 0:64, odd heads -> col 128:192.
                pvr = pv[:].rearrange("p (j par c) -> p j par c",
                                      par=2, c=HD)
                bvr = bv_bc[:, hf * half:(hf + 1) * half].rearrange(
                    "p (j par c) -> p j par c", par=2, c=HD)
                for par in range(2):
                    nc.vector.tensor_tensor(
                        out=vv3[:, tw, 3 * hf:3 * hf + 3,
                                2 * HD * par:2 * HD * par + HD],
                        in0=pvr[:, :, par, :], in1=bvr[:, :, par, :],
                        op=ALU.add)

    ht_cm.__exit__(None, None, None)

    # ---------------- phase C: attention ----------------
    # FFN weights for phases D/E prefetch during attention compute.
    wo_sb = wffnA.tile([128, ND, D], BF16, tag="wo_sb")
    for a in range(ND):
        nc.gpsimd.dma_start(out=wo_sb[:, a, :],
                            in_=w_o[a * 128:(a + 1) * 128, :])
    w1_sb = wffnA.tile([128, ND, I], BF16, tag="w1_sb")
    for d in range(ND):
        nc.gpsimd.dma_start(out=w1_sb[:, d, :],
                            in_=w1[d * 128:(d + 1) * 128, :])
    b1_sb = wffnA.tile([128, NI], F32, tag="b1_sb")
    nc.gpsimd.dma_start(
        out=b1_sb[:],
        in_=bass.AP(tensor=b1.tensor, offset=b1.offset,
                    ap=[[1, 128], [128, NI]]))

    attnT = at_pool.tile([128, ND, q_len], BF16, tag="attnT")

    with contextlib.ExitStack() as phC:
        sps = phC.enter_context(tc.tile_pool(name="sps", bufs=3, space="PSUM"))
        ops_ = phC.enter_context(
            tc.tile_pool(name="ops", bufs=2, space="PSUM"))
        epool = phC.enter_context(tc.tile_pool(name="epool", bufs=4))
        rpool = phC.enter_context(tc.tile_pool(name="rpool", bufs=2))

        for h in range(NH):
            hb2, hp = 64 * (h % 2), h // 2
            # attn output partitions = v-column half: low for even heads,
            # high for odd (shared-ones layout flips the halves).
            alo = 0 if h % 2 == 0 else 64
            dlo = 64 - alo
            for (q0, w, nfull) in attn_spans(q_start, q_len):
                po = ops_.tile([128, 512], F32, tag="po")
                ndiag = w // 128
                ktot = nfull + ndiag

                def scores_mm(ps_slice, kt, c0):
                    nc.tensor.matmul(
                        ps_slice,
                        kT2[hb2:hb2 + 64, hp, kt * 128:(kt + 1) * 128],
                        qT2[hb2:hb2 + 64, hp,
                            q0 + c0 - q_start:q0 + w - q_start],
                        start=True, stop=True)

                def pv_mm(kt, et_slice, c0):
                    nc.tensor.matmul(
                        po[:, c0:w], vaug_lhsT(kt, h), et_slice,
                        start=(kt == 0), stop=(kt == ktot - 1))

                # full key-tiles, two per PSUM bank (bf16 scores) so each
                # ACT exp covers 2*w elements (amortizes the 352-cyc fixed
                # ACTIVATE overhead)
                kt = 0
                while kt < nfull:
                    npair = min(2, nfull - kt)
                    ps = sps.tile([128, 2, 512], F32, tag="ps")
                    for jj in range(npair):
                        scores_mm(ps[:, jj, 0:w], kt + jj, 0)
                    et = epool.tile([128, 2, 512], BF16, tag="et")
                    nc.scalar.activation(out=et[:, 0:npair, 0:w],
                                         in_=ps[:, 0:npair, 0:w],
                                         func=AF.Exp, bias=0.0, scale=1.0 / HD)
                    for jj in range(npair):
                        pv_mm(kt + jj, et[:, jj, 0:w], 0)
                    kt += npair
                # diagonal key-tiles: causal handled by zeroing the exp'd
                # scores with a 0/1 bf16 mask on the first 128 query cols
                for diag_j in range(ndiag):
                    kt = nfull + diag_j
                    c0 = 128 * diag_j
                    wj = w - c0
                    ps = sps.tile([128, 2, 512], F32, tag="ps")
                    scores_mm(ps[:, 0, 0:wj], kt, c0)
                    et = epool.tile([128, 2, 512], BF16, tag="et")
                    nc.scalar.activation(out=et[:, 0, 0:wj], in_=ps[:, 0, 0:wj],
                                         func=AF.Exp, bias=0.0, scale=1.0 / HD)
                    nc.vector.tensor_tensor(
                        out=et[:, 0, 0:128], in0=et[:, 0, 0:128],
                        in1=causal01[:], op=ALU.mult)
                    pv_mm(kt, et[:, 0, 0:wj], c0)
                # 1/den = exp(-ln(den)): same ACT table set as the exps.
                lnd = rpool.tile([64, 512], F32, tag="lnd")
                nc.scalar.activation(out=lnd[:, 0:w],
                                     in_=po[dlo:dlo + 64, 0:w],
                                     func=AF.Ln, bias=0.0, scale=1.0)
                rcp = rpool.tile([64, 512], F32, tag="rcp")
                nc.scalar.activation(out=rcp[:, 0:w], in_=lnd[:, 0:w],
                                     func=AF.Exp, bias=0.0, scale=-1.0)
                nc.vector.tensor_tensor(
                    out=attnT[hb2:hb2 + 64, hp, q0 - q_start:q0 - q_start + w],
                    in0=po[alo:alo + 64, 0:w], in1=rcp[:, 0:w], op=ALU.mult)

    qkv_cm.__exit__(None, None, None)
    wearly_cm.__exit__(None, None, None)

    # ---------------- phase D: w_o + residual + LN2 + transpose ----------
    wffnB_cm = tc.tile_pool(name="wffnB", bufs=1)
    wffnB = wffnB_cm.__enter__()
    w2_sb = wffnB.tile([128, NI, D], BF16, tag="w2_sb")
    for i_ in range(NI):
        nc.gpsimd.dma_start(out=w2_sb[:, i_, :],
                            in_=w2[i_ * 128:(i_ + 1) * 128, :])
    oa_all = wffnB.tile([128, NT_Q, D], F32, tag="oa_all")
    h2T = wffnB.tile([128, ND, q_len], BF16, tag="h2T")
    Dh = D // 2

    with contextlib.ExitStack() as phD:
        wops = phD.enter_context(
            tc.tile_pool(name="wops", bufs=2, space="PSUM"))
        tpp2 = phD.enter_context(
            tc.tile_pool(name="tpp2", bufs=2, space="PSUM"))
        xpool2 = phD.enter_context(tc.tile_pool(name="xpool2", bufs=2))
        lnp2 = phD.enter_context(tc.tile_pool(name="lnp2", bufs=2))
        for tw in range(NT_Q):
            xo = xpool2.tile([128, D], F32, tag="xo")
            nc.sync.dma_start(
                out=xo[:],
                in_=x[q_start + tw * 128:q_start + (tw + 1) * 128, :])
            nc.vector.tensor_tensor(out=xo[:], in0=xo[:], in1=bo_bc[:],
                                    op=ALU.add)
            for hf in range(2):
                pw = wops.tile([128, Dh], F32, tag="pw")
                for a in range(ND):
                    nc.tensor.matmul(
                        pw[:], attnT[:, a, tw * 128:(tw + 1) * 128],
                        wo_sb[:, a, hf * Dh:(hf + 1) * Dh],
                        start=(a == 0), stop=(a == ND - 1))
                nc.vector.tensor_tensor(
                    out=oa_all[:, tw, hf * Dh:(hf + 1) * Dh], in0=pw[:],
                    in1=xo[:, hf * Dh:(hf + 1) * Dh], op=ALU.add)
            hb = layernorm_tokmajor(oa_all[:, tw, :], g2_bc, bl2_bc,
                                    lnp2, "ln2")
            for d in range(ND):
                pt = tpp2.tile([128, 128], BF16, tag="tp2_ps")
                nc.tensor.transpose(pt[:], hb[:, d * 128:(d + 1) * 128],
                                    ident[:])
                nc.vector.tensor_copy(
                    out=h2T[:, d, tw * 128:(tw + 1) * 128], in_=pt[:])

    # ---------------- phase E: FFN + gelu + residual -> out --------------
    with contextlib.ExitStack() as phE:
        f1ps = phE.enter_context(
            tc.tile_pool(name="f1ps", bufs=2, space="PSUM"))
        f2ps = phE.enter_context(
            tc.tile_pool(name="f2ps", bufs=2, space="PSUM"))
        opool = phE.enter_context(tc.tile_pool(name="opool", bufs=2))

        for c0 in range(0, q_len, 512):
            cw = min(512, q_len - c0)
            ctws = cw // 128
            ff1 = wffnB.tile([128, NI, 512], BF16, tag="ff1")
            for i_ in range(NI):
                pf = f1ps.tile([128, 512], F32, tag="pf1")
                for d in range(ND):
                    nc.tensor.matmul(
                        pf[:, 0:cw], w1_sb[:, d, i_ * 128:(i_ + 1) * 128],
                        h2T[:, d, c0:c0 + cw],
                        start=(d == 0), stop=(d == ND - 1))
                # evac on ACT (bias folded); DVE is loaded with gelu adds
                nc.scalar.activation(
                    out=ff1[:, i_, 0:cw], in_=pf[:, 0:cw],
                    func=AF.Identity, bias=b1_sb[:, i_:i_ + 1], scale=1.0)
            for twl in range(ctws):
                tw = c0 // 128 + twl
                ot = opool.tile([128, D], F32, tag="ot")
                for hf in range(2):
                    pg = f2ps.tile([128, Dh], F32, tag="pf2")
                    for i_ in range(NI):
                        nc.tensor.matmul(
                            pg[:], ff1[:, i_, twl * 128:(twl + 1) * 128],
                            w2_sb[:, i_, hf * Dh:(hf + 1) * Dh],
                            start=(i_ == 0), stop=(i_ == NI - 1))
                    sl = slice(hf * Dh, (hf + 1) * Dh)
                    gb = opool.tile([128, Dh], F32, tag="gb")
                    nc.vector.tensor_tensor(out=gb[:], in0=pg[:],
                                            in1=b2_bc[:, sl], op=ALU.add)
                    nc.scalar.activation(out=gb[:], in_=gb[:], func=AF.Gelu,
                                         bias=0.0, scale=1.0)
                    nc.vector.tensor_tensor(out=ot[:, sl], in0=gb[:],
                                            in1=oa_all[:, tw, sl],
                                            op=ALU.add)
                nc.sync.dma_start(out=out[tw * 128:(tw + 1) * 128, :],
                                  in_=ot[:])

    wffnB_cm.__exit__(None, None, None)


_NC_CACHE = {}


def build_kernel():
    key = (B, L, D, H, I, T0)
    if key in _NC_CACHE:
        return _NC_CACHE[key]
    nc = bacc.Bacc("TRN2", target_bir_lowering=False, debug=False,
                   num_devices=N_CORES)
    x = nc.dram_tensor("x", [L, D], F32, kind="ExternalInput").ap()
    w_qkv = nc.dram_tensor("w_qkv", [D, 3 * D], F32, kind="ExternalInput").ap()
    w_o = nc.dram_tensor("w_o", [D, D], F32, kind="ExternalInput").ap()
    w1 = nc.dram_tensor("w1", [D, I], F32, kind="ExternalInput").ap()
    w2 = nc.dram_tensor("w2", [I, D], F32, kind="ExternalInput").ap()
    bqkv = nc.dram_tensor("b_qkv", [3 * D], F32, kind="ExternalInput").ap()
    bo = nc.dram_tensor("b_o", [D], F32, kind="ExternalInput").ap()
    b1 = nc.dram_tensor("b1", [I], F32, kind="ExternalInput").ap()
    b2 = nc.dram_tensor("b2", [D], F32, kind="ExternalInput").ap()
    g1 = nc.dram_tensor("ln1_g", [D], F32, kind="ExternalInput").ap()
    bl1 = nc.dram_tensor("ln1_b", [D], F32, kind="ExternalInput").ap()
    g2 = nc.dram_tensor("ln2_g", [D], F32, kind="ExternalInput").ap()
    bl2 = nc.dram_tensor("ln2_b", [D], F32, kind="ExternalInput").ap()
    out = nc.dram_tensor("out", [T0, D], F32, kind="ExternalOutput").ap()
    io = (x, w_qkv, w_o, w1, w2, bqkv, bo, b1, b2, g1, bl1, g2, bl2, out)

    pid = nc.partition_id()
    with tile.TileContext(nc) as tc:
        with tc.If(pid % 2 == 0):
            with contextlib.ExitStack() as c0:
                build_body(nc, tc, c0, io, 0, T0, T0)
        with tc.If(pid % 2 == 1):
            with contextlib.ExitStack() as c1:
                build_body(nc, tc, c1, io, T0, T1, L)
    nc.compile()
    _NC_CACHE[key] = nc
    return nc


def kernel(**inputs):
    x = np.asarray(inputs["x"], dtype=np.float32)
    am = np.asarray(inputs["attention_mask"])
    assert am.all(), "kernel assumes attention_mask all-True (spec fill=ones)"
    names = ["w_qkv", "b_qkv", "w_o", "b_o", "w1", "b1", "w2", "b2",
             "ln1_g", "ln1_b", "ln2_g", "ln2_b"]
    common = {n: np.ascontiguousarray(np.asarray(inputs[n], np.float32))
              for n in names}
    nc = build_kernel()
    in_maps = []
    for pid in range(N_CORES):
        b = pid // 2
        m = dict(common)
        m["x"] = np.ascontiguousarray(x[b])
        in_maps.append(m)
    res = run_bass_kernel_spmd(nc, in_maps, core_ids=list(range(N_CORES)))
    out = np.empty((B, L, D), np.float32)
    for b in range(B):
        out[b, :T0] = res.results[2 * b]["out"][:T0]
        out[b, T0:] = res.results[2 * b + 1]["out"][:T1]
    return out


if __name__ == "__main__":
    rng = np.random.default_rng(0)
    ins = {
        "x": rng.standard_normal((B, L, D)).astype(np.float32),
        "attention_mask": np.ones((B, L), bool),
        "ln1_g": np.ones(D, np.float32), "ln1_b": np.zeros(D, np.float32),
        "w_qkv": (rng.standard_normal((D, 3 * D)) * 0.02).astype(np.float32),
        "b_qkv": np.zeros(3 * D, np.float32),
        "w_o": (rng.standard_normal((D, D)) * 0.02).astype(np.float32),
        "b_o": np.zeros(D, np.float32),
        "ln2_g": np.ones(D, np.float32), "ln2_b": np.zeros(D, np.float32),
        "w1": (rng.standard_normal((D, I)) * 0.02).astype(np.float32),
        "b1": np.zeros(I, np.float32),
        "w2": (rng.standard_normal((I, D)) * 0.02).astype(np.float32),
        "b2": np.zeros(D, np.float32),
    }
    o = kernel(**ins)
    print("kernel out:", o.shape, o.dtype, np.abs(o).max())
